# revision 2
# baseline (speedup 1.0000x reference)
"""MoE layer (7 routed top-2 + 1 shared) on 8 trn2 cores: top-2 sparse expert compute on 8 trn2 cores (data-parallel).

v2 -> v3 (driven by CoreSim timeline profile):
- Combine switched to NON-transpose dma_gather from an HBM expert-output
  buffer (token-major result). Transpose-mode SBUF gathers cost ~35ns/idx of
  Q7 descriptor-gen (4 x 17.9us serial, gated the whole tail); non-transpose
  is ~6.6ns/idx and the token-major layout kills the gating-broadcast
  machinery (per-partition scalar scale instead).
- Shared L2, final combine, and the output all token-major.
- Startup reordered: router x loads issue first; the dispatch-buffer zero DMA
  uses a 4-row source (672 descriptors instead of 2688) and issues after.
- Weight DMAs consolidated (w2 full-expert tiles, 7 DMAs instead of 56).
"""

import os
import sys

for _p in ("/opt/trn_rl_repo", "/root/.axon_site/_ro/trn_rl_repo"):
    if os.path.isdir(_p) and _p not in sys.path:
        sys.path.append(_p)

import numpy as np

import concourse.bacc as bacc
import concourse.bass as bass
import concourse.mybir as mybir
import concourse.tile as tile
from concourse import bass_utils
from concourse.masks import make_identity as masks_make_identity

# Patch walrus to honor --enable-ldw-opt (dedups back-to-back LDWEIGHTS for
# matmuls sharing a stationary operand; concourse disables it by default).
_LDW_OPT = False
_orig_run_command = bass_utils.run_command


def _run_command_ldw(argv, **kw):
    if _LDW_OPT:
        argv = ["--enable-ldw-opt=true" if a == "--enable-ldw-opt=false" else a
                for a in argv]
    return _orig_run_command(argv, **kw)


bass_utils.run_command = _run_command_ldw

B, S, D, H = 4, 2048, 1024, 1024
E = 7
N_CORES = 8
T = B * S
TC = T // N_CORES
P = 128
DC = D // P
HC = H // P
NTCH = TC // P
CAP = 384
SLOTS = E * CAP      # 2688
SRANKS = SLOTS // P  # 21

F32 = mybir.dt.float32
F16 = mybir.dt.float16
I16 = mybir.dt.int16


def build_nc(n_reps: int = 1, act_fn=None):
    if act_fn is None:
        act_fn = mybir.ActivationFunctionType.Gelu
    nc = bacc.Bacc("TRN2", target_bir_lowering=False, debug=False)

    # partition-contiguous layouts: [p, ...] with everything after p contiguous
    # in HBM, so each DMA is 128 big descriptors instead of 1024 small ones.
    xTp = nc.dram_tensor("xTp", (P, DC, TC), F32, kind="ExternalInput")
    xtokp = nc.dram_tensor("xtokp", (P, NTCH, D), F16, kind="ExternalInput")
    gwT = nc.dram_tensor("gwT", (D, E), F32, kind="ExternalInput")
    eb = nc.dram_tensor("eb", (E,), F32, kind="ExternalInput")
    rev7 = nc.dram_tensor("rev7", (E,), F32, kind="ExternalInput")
    trio = nc.dram_tensor("trio", (P, 2 * P), F32, kind="ExternalInput")
    capb = nc.dram_tensor("capb", (E,), F32, kind="ExternalInput")
    sw1q = nc.dram_tensor("sw1q", (P, H // 256, DC, 256), F16,
                          kind="ExternalInput")
    sb1 = nc.dram_tensor("sb1", (H,), F32, kind="ExternalInput")
    sw2q = nc.dram_tensor("sw2q", (P, HC, D), F16, kind="ExternalInput")
    sb2 = nc.dram_tensor("sb2", (D,), F32, kind="ExternalInput")
    rw1q = nc.dram_tensor("rw1q", (E, P, H // 256, DC, 256), F16,
                          kind="ExternalInput")
    rb1 = nc.dram_tensor("rb1", (E, H), F32, kind="ExternalInput")
    rw2q = nc.dram_tensor("rw2q", (E, P, HC, D), F16, kind="ExternalInput")
    rb2 = nc.dram_tensor("rb2", (E, D), F32, kind="ExternalInput")
    xdisp = nc.dram_tensor("xdisp", (SLOTS, D), F16, kind="ExternalOutput")
    eoh = nc.dram_tensor("eoh", (SLOTS, D), F16, kind="ExternalOutput")
    lscr = nc.dram_tensor("lscr", (2, TC), I16, kind="ExternalOutput")
    outt = nc.dram_tensor("outt", (TC, D), F32, kind="ExternalOutput")

    gwT_t = gwT.rearrange("(c p) e -> p c e", p=P)
    sb1_t = sb1.rearrange("(c p) -> p c", p=P)
    rb1_t = rb1.rearrange("e (c p) -> p e c", p=P)
    xdisp_t = xdisp.rearrange("(b p) d -> p b d", p=P)
    outt_t = outt.rearrange("(b p) d -> p b d", p=P)
    eoh_t = eoh.rearrange("(b p) d -> p b d", p=P)

    with tile.TileContext(nc) as tc:
        with (
            tc.tile_pool(name="const", bufs=1) as constp,
            tc.tile_pool(name="xc", bufs=2) as xcp,
            tc.tile_pool(name="xr", bufs=1) as xrp,
            tc.tile_pool(name="tkc", bufs=2) as tkcp,
            tc.tile_pool(name="hsh", bufs=1) as hshp,
            tc.tile_pool(name="xg", bufs=2) as xgp,
            tc.tile_pool(name="h", bufs=2) as hp,
            tc.tile_pool(name="eos", bufs=1) as eosp,
            tc.tile_pool(name="w1", bufs=2) as w1p,
            tc.tile_pool(name="w2", bufs=2) as w2p,
            tc.tile_pool(name="sw2f", bufs=1) as sw2fp,
            tc.tile_pool(name="fin", bufs=2) as finp,
            tc.tile_pool(name="rt", bufs=1) as rtp,
            tc.tile_pool(name="idx", bufs=1) as idxp,
            tc.tile_pool(name="pss", bufs=2, space="PSUM") as pssp,
            tc.tile_pool(name="psb", bufs=3, space="PSUM") as psbp,
        ):
            def body(_iv=None):
                # ---------- router-critical loads first (SP FIFO order) ----------
                gw_sb = constp.tile([P, DC, E], F32, tag="gw")
                nc.sync.dma_start(gw_sb[:], gwT_t[:])
                eb_sb = constp.tile([E, 1], F32, tag="eb")
                nc.sync.dma_start(eb_sb[:], eb[:, None])
                rev_sb = constp.tile([P, E], F32, tag="rev")
                nc.sync.dma_start(rev_sb[:], rev7[None, :].to_broadcast((P, E)))
                trio_sb = constp.tile([P, 2 * P], F32, tag="trio")
                nc.sync.dma_start(trio_sb[:], trio[:])
                capb_sb = constp.tile([E, 1], F32, tag="capb")
                nc.sync.dma_start(capb_sb[:], capb[:, None])
                ident = constp.tile([P, P], F32, tag="ident")
                masks_make_identity(nc, ident[:])
                if _LDW_OPT:
                    # distinct BIR hash for the ldw-opt NEFF cache entry
                    nc.vector.memset(ident[0:1, 0:1], 1.0)

                # x fp16 feature-major: cast on ACT (idle early; keeps DVE free)
                xr = xrp.tile([P, DC, TC], F16, tag="xr")
                ps_lf = [pssp.tile([E, 512], F32, tag="ps", name=f"psl{h}")
                         for h in range(2)]
                for cg in range(4):
                    xc = xcp.tile([P, 2, TC], F32, tag="xc")
                    nc.sync.dma_start(xc[:], xTp[:, 2 * cg:2 * cg + 2, :])
                    for dcl in range(2):
                        dc = 2 * cg + dcl
                        for tb in range(2):
                            nc.tensor.matmul(
                                ps_lf[tb][:],
                                gw_sb[:, dc, :],
                                xc[:, dcl, tb * 512:(tb + 1) * 512],
                                start=(dc == 0), stop=(dc == DC - 1),
                            )
                    nc.scalar.activation(
                        xr[:, 2 * cg:2 * cg + 2, :], xc[:],
                        mybir.ActivationFunctionType.Copy)

                # ---------- remaining constants ----------
                sb1_sb = constp.tile([P, HC], F32, tag="sb1")
                nc.sync.dma_start(sb1_sb[:], sb1_t[:])
                rb1_sb = constp.tile([P, E, HC], F32, tag="rb1")
                nc.sync.dma_start(rb1_sb[:], rb1_t[:])
                zer = constp.tile([P, 3 * D], F16, tag="zer")
                nc.vector.memset(zer[:], 0.0)
                # zero dispatch buffer (896 descriptors of 6KB)
                nc.sync.dma_start(
                    xdisp.rearrange("(a p f) d -> p a (f d)", p=P, f=3),
                    zer[:, None, :].to_broadcast((P, SLOTS // (P * 3), 3 * D)))
                # token-major x (scatter source), partition-contiguous layout
                xtk = tkcp.tile([P, NTCH, D], F16, tag="tkc")
                nc.sync.dma_start(xtk[:], xtokp[:])

                lgT = rtp.tile([E, TC], F32, tag="efm", name="lgT")
                for h in range(2):
                    nc.vector.tensor_scalar_add(
                        lgT[:, h * 512:(h + 1) * 512], ps_lf[h][:], eb_sb[:])

                # ---------- router top-2 (token-major, fp32) ----------
                def rt3(tag):
                    return rtp.tile([P, NTCH, E], F32, tag=tag, name=tag)

                def rt1(tag):
                    return rtp.tile([P, NTCH, 1], F32, tag=tag, name=tag)

                def bc3(t):
                    return t[:].to_broadcast((P, NTCH, E))

                rev3 = rev_sb[:, None, :].to_broadcast((P, NTCH, E))

                lg = rt3("lg")
                for tch in range(NTCH):
                    pt = pssp.tile([P, E], F32, tag="ps", name="pt")
                    nc.tensor.transpose(pt[:], lgT[:, tch * P:(tch + 1) * P],
                                        ident[0:E, 0:E])
                    nc.vector.tensor_copy(lg[:, tch, :], pt[:])
                m1 = rt1("m1")
                nc.vector.reduce_max(m1[:], lg[:], axis=mybir.AxisListType.X)
                mask1 = rt3("mask1")
                nc.vector.tensor_tensor(mask1[:], lg[:], bc3(m1),
                                        op=mybir.AluOpType.is_equal)
                mv1 = rt3("mv1")
                nc.vector.tensor_tensor(mv1[:], mask1[:], rev3,
                                        op=mybir.AluOpType.mult)
                sel1 = rt1("sel1")
                nc.vector.reduce_max(sel1[:], mv1[:], axis=mybir.AxisListType.X)
                m1f = rt3("m1f")
                nc.vector.tensor_tensor(m1f[:], mv1[:], bc3(sel1),
                                        op=mybir.AluOpType.is_equal)
                l2 = rt3("l2")
                nc.vector.tensor_scalar(l2[:], m1f[:], -1.0e30, None,
                                        op0=mybir.AluOpType.mult)
                nc.vector.tensor_add(l2[:], l2[:], lg[:])
                m2 = rt1("m2")
                nc.vector.reduce_max(m2[:], l2[:], axis=mybir.AxisListType.X)
                mask2 = rt3("mask2")
                nc.vector.tensor_tensor(mask2[:], l2[:], bc3(m2),
                                        op=mybir.AluOpType.is_equal)
                mv2 = rt3("mv2")
                nc.vector.tensor_tensor(mv2[:], mask2[:], rev3,
                                        op=mybir.AluOpType.mult)
                sel2 = rt1("sel2")
                nc.vector.reduce_max(sel2[:], mv2[:], axis=mybir.AxisListType.X)
                m2f = rt3("m2f")
                nc.vector.tensor_tensor(m2f[:], mv2[:], bc3(sel2),
                                        op=mybir.AluOpType.is_equal)
                dlt = rt1("dlt")
                nc.vector.tensor_sub(dlt[:], m2[:], m1[:])
                ex = rt1("ex")
                nc.scalar.activation(ex[:], dlt[:],
                                     mybir.ActivationFunctionType.Exp)
                den = rt1("den")
                nc.vector.tensor_scalar_add(den[:], ex[:], 1.0)
                w1t = rt1("w1t")
                nc.vector.reciprocal(w1t[:], den[:])
                w2t = rt1("w2t")
                nc.vector.tensor_mul(w2t[:], ex[:], w1t[:])
                comb3 = rt3("comb3")
                nc.vector.tensor_tensor(comb3[:], m1f[:], bc3(w1t),
                                        op=mybir.AluOpType.mult)
                m2fw = rt3("m2fw")
                nc.vector.tensor_tensor(m2fw[:], m2f[:], bc3(w2t),
                                        op=mybir.AluOpType.mult)
                nc.vector.tensor_add(comb3[:], comb3[:], m2fw[:])
                sel3 = rt3("sel3")
                nc.vector.tensor_add(sel3[:], m1f[:], m2f[:])
                wh = rtp.tile([P, NTCH, 2], F16, tag="wh", name="wh")
                nc.vector.tensor_copy(wh[:, :, 0], w1t[:, :, 0])
                nc.vector.tensor_copy(wh[:, :, 1], w2t[:, :, 0])

                # comb feature-major rows (fp16) for the rb2/sb2 fold
                cr8 = rtp.tile([8, TC], F16, tag="cr8", name="cr8")
                nc.vector.memset(cr8[:], 1.0)  # row 7 stays all-ones
                for tch in range(NTCH):
                    pt2 = pssp.tile([E, P], F32, tag="ps", name="pt2")
                    nc.tensor.transpose(pt2[:], comb3[:, tch, :], ident[:])
                    nc.vector.tensor_copy(cr8[0:E, tch * P:(tch + 1) * P], pt2[:])

                # ---------- slot assignment (cumsum via PE) ----------
                slotFM = rtp.tile([E, TC], F32, tag="efm", name="slotFM")
                for h in range(2):
                    pos_ps = pssp.tile([E, 512], F32, tag="ps", name=f"pos{h}")
                    for tbl in range(4):
                        tb = h * 4 + tbl
                        for tpb in range(tb + 1):
                            rhs = (trio_sb[:, 0:P] if tpb == tb
                                   else trio_sb[:, P:2 * P])
                            nc.tensor.matmul(
                                pos_ps[:, tbl * P:(tbl + 1) * P],
                                sel3[:, tpb, :],
                                rhs,
                                start=(tpb == 0), stop=(tpb == tb),
                            )
                    nc.vector.tensor_scalar_add(
                        slotFM[:, h * 512:(h + 1) * 512], pos_ps[:], capb_sb[:])
                slot3 = rt3("slot3")
                for tch in range(NTCH):
                    pt3 = pssp.tile([P, E], F32, tag="ps", name="pt3")
                    nc.tensor.transpose(pt3[:], slotFM[:, tch * P:(tch + 1) * P],
                                        ident[0:E, 0:E])
                    nc.vector.tensor_copy(slot3[:, tch, :], pt3[:])
                tt = rt3("tt")
                nc.vector.tensor_tensor(tt[:], m1f[:], slot3[:],
                                        op=mybir.AluOpType.mult)
                slot1 = rt1("slot1")
                nc.vector.reduce_sum(slot1[:], tt[:], axis=mybir.AxisListType.X)
                nc.vector.tensor_tensor(tt[:], m2f[:], slot3[:],
                                        op=mybir.AluOpType.mult)
                slot2 = rt1("slot2")
                nc.vector.reduce_sum(slot2[:], tt[:], axis=mybir.AxisListType.X)

                s12 = idxp.tile([P, 2, NTCH], I16, tag="s12")
                nc.vector.tensor_copy(s12[:, 0, :], slot1[:, :, 0])
                nc.vector.tensor_copy(s12[:, 1, :], slot2[:, :, 0])
                nc.gpsimd.dma_start(
                    lscr.rearrange("k (c p) -> p k c", p=P), s12[:])
                wr = idxp.tile([P, 2, TC // 16], I16, tag="wr")
                for r in range(8):
                    nc.gpsimd.dma_start(
                        wr[r * 16:(r + 1) * 16, :, :],
                        lscr.rearrange("k (c q) -> q k c", q=16))

                # ---------- dispatch scatters (SWDGE) ----------
                for k in range(2):
                    nc.gpsimd.dma_scatter_add(
                        xdisp[:, :], xtk[:], wr[:, k, :],
                        TC, TC, D,
                    )

                # tail-only constants (issued late, off the critical path)
                rbf = finp.tile([E, D], F32, tag="fin", name="rbf")
                nc.sync.dma_start(rbf[:], rb2[:])
                sbf = finp.tile([1, D], F32, tag="fin", name="sbf")
                nc.sync.dma_start(sbf[:], sb2[None, :])
                cb8 = constp.tile([8, D], F16, tag="cb8")
                nc.vector.tensor_copy(cb8[0:E, :], rbf[:])
                sbh = constp.tile([1, D], F16, tag="sbh")
                nc.vector.tensor_copy(sbh[:], sbf[:])
                nc.sync.dma_start(cb8[E:E + 1, :], sbh[0:1, :])

                # ---------- shared expert L1 (fp16, feature-major) ----------
                hsh = hshp.tile([P, HC, TC], F16, tag="hsh")
                WCH = 256
                for cg in range(2):
                    wt = w1p.tile([P, 2, DC, WCH], F16, tag="w1")
                    nc.sync.dma_start(wt[:], sw1q[:, 2 * cg:2 * cg + 2, :, :])
                    for cil in range(2):
                        for hl in range(WCH // P):
                            hcc = (2 * cg + cil) * 2 + hl
                            ph = psbp.tile([P, 2, 512], F32, tag="psb")
                            for dc in range(DC):
                                for tb in range(2):
                                    nc.tensor.matmul(
                                        ph[:, tb, :],
                                        wt[:, cil, dc, hl * P:(hl + 1) * P],
                                        xr[:, dc, tb * 512:(tb + 1) * 512],
                                        start=(dc == 0), stop=(dc == DC - 1),
                                    )
                            nc.scalar.activation(
                                hsh[:, hcc, :],
                                ph[:].rearrange("p a b -> p (a b)"),
                                act_fn, bias=sb1_sb[:, hcc:hcc + 1])
                # shared weights for the tail L2 (load early, big DMA)
                sw2f = sw2fp.tile([P, HC, D], F16, tag="sw2f")
                nc.sync.dma_start(sw2f[:], sw2q[:])

                # ---------- routed experts ----------
                for e in range(E):
                    # one xbar-transpose load for the whole expert block:
                    # xdisp columns are stored permuted (col k <-> feature
                    # (k%8)*128 + k//8) so the transposed [128, 8, CAP] output
                    # lands exactly feature-major.
                    xg = xgp.tile([P, DC, CAP], F16, tag="xg")
                    nc.sync.dma_start_transpose(
                        xg[:], xdisp[e * CAP:(e + 1) * CAP, :])
                    hbuf = hp.tile([P, HC, CAP], F16, tag="h")
                    for cg in range(2):
                        wt = w1p.tile([P, 2, DC, WCH], F16, tag="w1")
                        nc.sync.dma_start(wt[:], rw1q[e][:, 2 * cg:2 * cg + 2,
                                                        :, :])
                        for cil in range(2):
                            for hl in range(WCH // P):
                                hcc = (2 * cg + cil) * 2 + hl
                                ph = pssp.tile([P, CAP], F32, tag="ps",
                                               name="ph")
                                for dc in range(DC):
                                    nc.tensor.matmul(
                                        ph[:],
                                        wt[:, cil, dc, hl * P:(hl + 1) * P],
                                        xg[:, dc, :],
                                        start=(dc == 0), stop=(dc == DC - 1),
                                    )
                                nc.scalar.activation(
                                    hbuf[:, hcc, :], ph[:],
                                    act_fn, bias=rb1_sb[:, e, hcc:hcc + 1])
                    # L2: slot-major, stationary = h slices, moving = rw2 rows
                    w2f = w2p.tile([P, HC, D], F16, tag="w2")
                    nc.sync.dma_start(w2f[:], rw2q[e][:, :, :])
                    pos = [psbp.tile([P, 2, 512], F32, tag="psb",
                                     name=f"pos{e}_{i}")
                           for i in range(CAP // P)]
                    for hcc in range(HC):
                        for sb_ in range(CAP // P):
                            for nh in range(2):
                                nc.tensor.matmul(
                                    pos[sb_][:, nh, :],
                                    hbuf[:, hcc, sb_ * P:(sb_ + 1) * P],
                                    w2f[:, hcc, nh * 512:(nh + 1) * 512],
                                    start=(hcc == 0), stop=(hcc == HC - 1),
                                )
                    eos = eosp.tile([P, CAP // P, D], F16, tag="eos")
                    for sb_ in range(CAP // P):
                        nc.vector.tensor_copy(
                            eos[:, sb_, :],
                            pos[sb_][:].rearrange("p a b -> p (a b)"))
                    nc.sync.dma_start(
                        eoh_t[:, e * (CAP // P):(e + 1) * (CAP // P), :], eos[:])

                # ---------- combine gathers (non-transpose, HBM source) ----------
                cc = [None, None]
                for k in range(2):
                    cc[k] = tkcp.tile([P, NTCH, D], F16, tag="tkc",
                                      name=f"cc{k}")
                    nc.gpsimd.dma_gather(
                        cc[k][:], eoh[:, :], wr[:, k, :],
                        TC, TC, D,
                    )

                # ---------- shared L2 (token-major) + final + out ----------
                # 1-bank psum tiles so this never waits on the expert-L2 psum
                # rotation; fin split per 512-wide half for pipelining.
                for tb in range(NTCH):
                    fin = finp.tile([P, D], F32, tag="fin")
                    for nh in range(2):
                        po = pssp.tile([P, 512], F32, tag="ps",
                                       name=f"po{tb}_{nh}")
                        for hcc in range(HC):
                            nc.tensor.matmul(
                                po[:],
                                hsh[:, hcc, tb * P:(tb + 1) * P],
                                sw2f[:, hcc, nh * 512:(nh + 1) * 512],
                                start=(hcc == 0), stop=False,
                            )
                        nc.tensor.matmul(
                            po[:],
                            cr8[:, tb * P:(tb + 1) * P],
                            cb8[:, nh * 512:(nh + 1) * 512],
                            start=False, stop=True,
                        )
                        sl = slice(nh * 512, (nh + 1) * 512)
                        nc.vector.tensor_tensor(
                            fin[:, sl], cc[0][:, tb, sl],
                            wh[:, tb, 0:1].to_broadcast((P, 512)),
                            op=mybir.AluOpType.mult)
                        fin2 = finp.tile([P, 512], F32, tag="fin2")
                        nc.vector.tensor_tensor(
                            fin2[:], cc[1][:, tb, sl],
                            wh[:, tb, 1:2].to_broadcast((P, 512)),
                            op=mybir.AluOpType.mult)
                        nc.vector.tensor_add(fin[:, sl], fin[:, sl], fin2[:])
                        nc.vector.tensor_add(fin[:, sl], fin[:, sl], po[:])
                    nc.sync.dma_start(outt_t[:, tb, :], fin[:])

            if n_reps == 1:
                body()
            else:
                tc.For_i_unrolled(0, n_reps, 1, body, max_unroll=1)

    nc.compile()
    return nc


class Runner:
    """Compile once, dispatch many times (axon/PJRT path)."""

    def __init__(self, nc):
        import jax
        from jax.sharding import Mesh, PartitionSpec
        from jax.experimental.shard_map import shard_map
        from concourse import bass2jax

        bass2jax.install_neuronx_cc_hook()
        self.nc = nc
        self.jax = jax
        pname = nc.partition_id_tensor.name if nc.partition_id_tensor else None
        in_names, out_names, out_avals = [], [], []
        for alloc in nc.m.functions[0].allocations:
            if not isinstance(alloc, mybir.MemoryLocationSet):
                continue
            name = alloc.memorylocations[0].name
            if alloc.kind == "ExternalInput":
                if name != pname:
                    in_names.append(name)
            elif alloc.kind == "ExternalOutput":
                out_names.append(name)
                out_avals.append(jax.core.ShapedArray(
                    tuple(alloc.tensor_shape), mybir.dt.np(alloc.dtype)))
        self.in_names, self.out_names, self.out_avals = in_names, out_names, out_avals
        all_names = in_names + out_names + ([pname] if pname else [])

        def _body(*args):
            operands = list(args)
            if pname is not None:
                operands.append(bass2jax.partition_id_tensor())
            outs = bass2jax._bass_exec_p.bind(
                *operands,
                out_avals=tuple(out_avals),
                in_names=tuple(all_names),
                out_names=tuple(out_names),
                lowering_input_output_aliases=(),
                sim_require_finite=True, sim_require_nnan=True, nc=nc)
            return tuple(outs)

        devices = jax.devices()[:N_CORES]
        mesh = Mesh(np.asarray(devices), ("core",))
        nin = len(in_names) + len(out_names)
        self.fn = jax.jit(
            shard_map(_body, mesh=mesh,
                      in_specs=(PartitionSpec("core"),) * nin,
                      out_specs=(PartitionSpec("core"),) * len(out_names),
                      check_rep=False),
            keep_unused=True)

    def concat_inputs(self, in_maps):
        args = []
        for name in self.in_names:
            args.append(np.concatenate([m[name] for m in in_maps], axis=0))
        for av in self.out_avals:
            args.append(np.zeros((N_CORES * av.shape[0],) + av.shape[1:],
                                 av.dtype))
        return args

    def __call__(self, args):
        outs = self.fn(*args)
        self.jax.block_until_ready(outs)
        return outs

    def split_outputs(self, outs):
        res = []
        for c in range(N_CORES):
            d = {}
            for i, name in enumerate(self.out_names):
                a = np.asarray(outs[i])
                d[name] = a.reshape(N_CORES, *self.out_avals[i].shape)[c]
            res.append(d)
        return res


_RUNNER_CACHE = {}


def get_runner(n_reps=1, act_fn=None):
    key = (n_reps, act_fn)
    if key not in _RUNNER_CACHE:
        _RUNNER_CACHE[key] = Runner(build_nc(n_reps, act_fn=act_fn))
    return _RUNNER_CACHE[key]


# xdisp column k holds feature (k%8)*128 + k//8, so the single xbar-transpose
# load of an expert block lands feature-major as [128(p), 8(c), CAP].
_XPERM = np.arange(D)  # identity: xbar 3D out flattens col = c*128+p


def make_in_maps(x, gate_w, expert_bias, sw1, sb1, sw2, sb2, rw1, rb1, rw2, rb2):
    xf = np.ascontiguousarray(np.asarray(x, dtype=np.float32).reshape(T, D))
    gwT = np.ascontiguousarray(np.asarray(gate_w, np.float32).T)
    rev = np.arange(E, 0, -1, dtype=np.float32)
    tri = np.triu(np.ones((P, P), np.float32), k=1)
    trio = np.concatenate([tri, np.ones((P, P), np.float32)], axis=1)
    capb = np.arange(E, dtype=np.float32) * CAP

    def h16(a):
        return np.ascontiguousarray(np.asarray(a, np.float32).astype(np.float16))

    def f32(a):
        return np.ascontiguousarray(np.asarray(a, np.float32))

    sw1_16 = np.asarray(sw1, np.float32).astype(np.float16)
    sw2_16 = np.asarray(sw2, np.float32).astype(np.float16)
    rw1_16 = np.asarray(rw1, np.float32).astype(np.float16)
    rw2_16 = np.asarray(rw2, np.float32).astype(np.float16)
    shared = {
        "gwT": gwT, "eb": f32(expert_bias), "rev7": rev,
        "trio": np.ascontiguousarray(trio), "capb": capb,
        "sw1q": np.ascontiguousarray(
            sw1_16.reshape(8, P, 4, 256).transpose(1, 2, 0, 3)),
        "sb1": f32(sb1),
        "sw2q": np.ascontiguousarray(
            sw2_16.reshape(8, P, D).transpose(1, 0, 2)),
        "sb2": f32(sb2),
        "rw1q": np.ascontiguousarray(
            rw1_16.reshape(E, 8, P, 4, 256).transpose(0, 2, 3, 1, 4)),
        "rb1": f32(rb1),
        "rw2q": np.ascontiguousarray(
            rw2_16.reshape(E, 8, P, D).transpose(0, 2, 1, 3)),
        "rb2": f32(rb2),
    }
    in_maps = []
    for c in range(N_CORES):
        xs = xf[c * TC:(c + 1) * TC, :]
        in_maps.append({
            "xTp": np.ascontiguousarray(
                xs.T.reshape(8, P, TC).transpose(1, 0, 2)),
            "xtokp": np.ascontiguousarray(
                h16(xs)[:, _XPERM].reshape(8, P, D).transpose(1, 0, 2)),
            **shared,
        })
    return in_maps


def kernel(x, gate_w, expert_bias, sw1, sb1, sw2, sb2, rw1, rb1, rw2, rb2):
    runner = get_runner(1)
    in_maps = make_in_maps(x, gate_w, expert_bias, sw1, sb1, sw2, sb2,
                           rw1, rb1, rw2, rb2)
    outs = runner(runner.concat_inputs(in_maps))
    res = runner.split_outputs(outs)
    parts = [res[c]["outt"] for c in range(N_CORES)]
    out = np.concatenate(parts, axis=0).reshape(B, S, D)
    return np.ascontiguousarray(out.astype(np.float32))


# revision 3
# speedup vs baseline: 1.0815x; 1.0815x over previous
"""MoE layer (7 routed top-2 + 1 shared) on 8 trn2 cores: top-2 sparse expert compute on 8 trn2 cores (data-parallel).

v2 -> v3 (driven by CoreSim timeline profile):
- Combine switched to NON-transpose dma_gather from an HBM expert-output
  buffer (token-major result). Transpose-mode SBUF gathers cost ~35ns/idx of
  Q7 descriptor-gen (4 x 17.9us serial, gated the whole tail); non-transpose
  is ~6.6ns/idx and the token-major layout kills the gating-broadcast
  machinery (per-partition scalar scale instead).
- Shared L2, final combine, and the output all token-major.
- Startup reordered: router x loads issue first; the dispatch-buffer zero DMA
  uses a 4-row source (672 descriptors instead of 2688) and issues after.
- Weight DMAs consolidated (w2 full-expert tiles, 7 DMAs instead of 56).
"""

import os
import sys

for _p in ("/opt/trn_rl_repo", "/root/.axon_site/_ro/trn_rl_repo"):
    if os.path.isdir(_p) and _p not in sys.path:
        sys.path.append(_p)

import numpy as np

import concourse.bacc as bacc
import concourse.bass as bass
import concourse.mybir as mybir
import concourse.tile as tile
from concourse import bass_utils
from concourse.masks import make_identity as masks_make_identity

# Patch walrus to honor --enable-ldw-opt (dedups back-to-back LDWEIGHTS for
# matmuls sharing a stationary operand; concourse disables it by default).
_LDW_OPT = False
_orig_run_command = bass_utils.run_command


def _run_command_ldw(argv, **kw):
    if _LDW_OPT:
        argv = ["--enable-ldw-opt=true" if a == "--enable-ldw-opt=false" else a
                for a in argv]
    return _orig_run_command(argv, **kw)


bass_utils.run_command = _run_command_ldw

B, S, D, H = 4, 2048, 1024, 1024
E = 7
N_CORES = 8
T = B * S
TC = T // N_CORES
P = 128
DC = D // P
HC = H // P
NTCH = TC // P
CAP = 384
SLOTS = E * CAP      # 2688
SRANKS = SLOTS // P  # 21

F32 = mybir.dt.float32
F16 = mybir.dt.float16
I16 = mybir.dt.int16


def build_nc(n_reps: int = 1, act_fn=None):
    if act_fn is None:
        act_fn = mybir.ActivationFunctionType.Gelu
    nc = bacc.Bacc("TRN2", target_bir_lowering=False, debug=False)

    # partition-contiguous layouts: [p, ...] with everything after p contiguous
    # in HBM, so each DMA is 128 big descriptors instead of 1024 small ones.
    xTp = nc.dram_tensor("xTp", (P, DC, TC), F32, kind="ExternalInput")
    xtokp = nc.dram_tensor("xtokp", (P, NTCH, D), F16, kind="ExternalInput")
    gwT = nc.dram_tensor("gwT", (D, E), F32, kind="ExternalInput")
    eb = nc.dram_tensor("eb", (E,), F32, kind="ExternalInput")
    rev7 = nc.dram_tensor("rev7", (E,), F32, kind="ExternalInput")
    trio = nc.dram_tensor("trio", (P, 2 * P), F32, kind="ExternalInput")
    capb = nc.dram_tensor("capb", (E,), F32, kind="ExternalInput")
    sw1q = nc.dram_tensor("sw1q", (P, H // 256, DC, 256), F16,
                          kind="ExternalInput")
    sb1 = nc.dram_tensor("sb1", (H,), F32, kind="ExternalInput")
    sw2q = nc.dram_tensor("sw2q", (P, HC, D), F16, kind="ExternalInput")
    sb2 = nc.dram_tensor("sb2", (D,), F32, kind="ExternalInput")
    rw1q = nc.dram_tensor("rw1q", (E, P, H // 256, DC, 256), F16,
                          kind="ExternalInput")
    rb1 = nc.dram_tensor("rb1", (E, H), F32, kind="ExternalInput")
    rw2q = nc.dram_tensor("rw2q", (E, P, HC, D), F16, kind="ExternalInput")
    rb2 = nc.dram_tensor("rb2", (E, D), F32, kind="ExternalInput")
    xdisp = nc.dram_tensor("xdisp", (SLOTS, D), F16, kind="ExternalOutput")
    eoh = nc.dram_tensor("eoh", (SLOTS, D), F16, kind="ExternalOutput")
    lscr = nc.dram_tensor("lscr", (2, TC), I16, kind="ExternalOutput")
    outt = nc.dram_tensor("outt", (TC, D), F32, kind="ExternalOutput")

    gwT_t = gwT.rearrange("(c p) e -> p c e", p=P)
    sb1_t = sb1.rearrange("(c p) -> p c", p=P)
    rb1_t = rb1.rearrange("e (c p) -> p e c", p=P)
    xdisp_t = xdisp.rearrange("(b p) d -> p b d", p=P)
    outt_t = outt.rearrange("(b p) d -> p b d", p=P)
    eoh_t = eoh.rearrange("(b p) d -> p b d", p=P)

    with tile.TileContext(nc) as tc:
        with (
            tc.tile_pool(name="const", bufs=1) as constp,
            tc.tile_pool(name="xc", bufs=2) as xcp,
            tc.tile_pool(name="xr", bufs=1) as xrp,
            tc.tile_pool(name="tkc", bufs=2) as tkcp,
            tc.tile_pool(name="hsh", bufs=1) as hshp,
            tc.tile_pool(name="xg", bufs=2) as xgp,
            tc.tile_pool(name="h", bufs=2) as hp,
            tc.tile_pool(name="eos", bufs=1) as eosp,
            tc.tile_pool(name="w1", bufs=2) as w1p,
            tc.tile_pool(name="w2", bufs=2) as w2p,
            tc.tile_pool(name="sw2f", bufs=1) as sw2fp,
            tc.tile_pool(name="fin", bufs=2) as finp,
            tc.tile_pool(name="rt", bufs=1) as rtp,
            tc.tile_pool(name="idx", bufs=1) as idxp,
            tc.tile_pool(name="pss", bufs=2, space="PSUM") as pssp,
            tc.tile_pool(name="psb", bufs=3, space="PSUM") as psbp,
        ):
            def body(_iv=None):
                # ---------- router-critical loads first (SP FIFO order) ----------
                gw_sb = constp.tile([P, DC, E], F32, tag="gw")
                nc.sync.dma_start(gw_sb[:], gwT_t[:])
                eb_sb = constp.tile([E, 1], F32, tag="eb")
                nc.sync.dma_start(eb_sb[:], eb[:, None])
                rev_sb = constp.tile([P, E], F32, tag="rev")
                nc.sync.dma_start(rev_sb[:], rev7[None, :].to_broadcast((P, E)))
                trio_sb = constp.tile([P, 2 * P], F32, tag="trio")
                nc.sync.dma_start(trio_sb[:], trio[:])
                capb_sb = constp.tile([E, 1], F32, tag="capb")
                nc.sync.dma_start(capb_sb[:], capb[:, None])
                ident = constp.tile([P, P], F32, tag="ident")
                masks_make_identity(nc, ident[:])
                if _LDW_OPT:
                    # distinct BIR hash for the ldw-opt NEFF cache entry
                    nc.vector.memset(ident[0:1, 0:1], 1.0)

                # x fp16 feature-major: cast on ACT (idle early; keeps DVE free)
                xr = xrp.tile([P, DC, TC], F16, tag="xr")
                ps_lf = [pssp.tile([E, 512], F32, tag="ps", name=f"psl{h}")
                         for h in range(2)]
                for cg in range(4):
                    xc = xcp.tile([P, 2, TC], F32, tag="xc")
                    nc.sync.dma_start(xc[:], xTp[:, 2 * cg:2 * cg + 2, :])
                    for dcl in range(2):
                        dc = 2 * cg + dcl
                        for tb in range(2):
                            nc.tensor.matmul(
                                ps_lf[tb][:],
                                gw_sb[:, dc, :],
                                xc[:, dcl, tb * 512:(tb + 1) * 512],
                                start=(dc == 0), stop=(dc == DC - 1),
                            )
                    nc.scalar.activation(
                        xr[:, 2 * cg:2 * cg + 2, :], xc[:],
                        mybir.ActivationFunctionType.Copy)

                # ---------- remaining constants ----------
                sb1_sb = constp.tile([P, HC], F32, tag="sb1")
                nc.sync.dma_start(sb1_sb[:], sb1_t[:])
                rb1_sb = constp.tile([P, E, HC], F32, tag="rb1")
                nc.sync.dma_start(rb1_sb[:], rb1_t[:])
                # preload shared-expert weights (SP queue, ahead of zero/xtk;
                # their matmuls are emitted later so PE order is unchanged)
                swt = []
                for cg in range(2):
                    wt = w1p.tile([P, 2, DC, 256], F16, tag="w1",
                                  name=f"swt{cg}")
                    nc.sync.dma_start(wt[:], sw1q[:, 2 * cg:2 * cg + 2, :, :])
                    swt.append(wt)
                sw2f = sw2fp.tile([P, HC, D], F16, tag="sw2f")
                nc.sync.dma_start(sw2f[:], sw2q[:])
                zer = constp.tile([P, 3 * D], F16, tag="zer")
                nc.vector.memset(zer[:], 0.0)
                # zero dispatch buffer (896 descriptors of 6KB)
                nc.sync.dma_start(
                    xdisp.rearrange("(a p f) d -> p a (f d)", p=P, f=3),
                    zer[:, None, :].to_broadcast((P, SLOTS // (P * 3), 3 * D)))
                # token-major x (scatter source), partition-contiguous layout
                xtk = tkcp.tile([P, NTCH, D], F16, tag="tkc")
                nc.sync.dma_start(xtk[:], xtokp[:])

                lgT = rtp.tile([E, TC], F32, tag="efm", name="lgT")
                for h in range(2):
                    nc.vector.tensor_scalar_add(
                        lgT[:, h * 512:(h + 1) * 512], ps_lf[h][:], eb_sb[:])

                # ---------- router top-2 (token-major, fp32) ----------
                def rt3(tag):
                    return rtp.tile([P, NTCH, E], F32, tag=tag, name=tag)

                def rt1(tag):
                    return rtp.tile([P, NTCH, 1], F32, tag=tag, name=tag)

                def bc3(t):
                    return t[:].to_broadcast((P, NTCH, E))

                rev3 = rev_sb[:, None, :].to_broadcast((P, NTCH, E))

                lg = rt3("lg")
                for tch in range(NTCH):
                    pt = pssp.tile([P, E], F32, tag="ps", name="pt")
                    nc.tensor.transpose(pt[:], lgT[:, tch * P:(tch + 1) * P],
                                        ident[0:E, 0:E])
                    nc.vector.tensor_copy(lg[:, tch, :], pt[:])
                m1 = rt1("m1")
                nc.vector.reduce_max(m1[:], lg[:], axis=mybir.AxisListType.X)
                mask1 = rt3("mask1")
                nc.vector.tensor_tensor(mask1[:], lg[:], bc3(m1),
                                        op=mybir.AluOpType.is_equal)
                mv1 = rt3("mv1")
                nc.vector.tensor_tensor(mv1[:], mask1[:], rev3,
                                        op=mybir.AluOpType.mult)
                sel1 = rt1("sel1")
                nc.vector.reduce_max(sel1[:], mv1[:], axis=mybir.AxisListType.X)
                m1f = rt3("m1f")
                nc.vector.tensor_tensor(m1f[:], mv1[:], bc3(sel1),
                                        op=mybir.AluOpType.is_equal)
                l2 = rt3("l2")
                nc.vector.tensor_scalar(l2[:], m1f[:], -1.0e30, None,
                                        op0=mybir.AluOpType.mult)
                nc.vector.tensor_add(l2[:], l2[:], lg[:])
                m2 = rt1("m2")
                nc.vector.reduce_max(m2[:], l2[:], axis=mybir.AxisListType.X)
                mask2 = rt3("mask2")
                nc.vector.tensor_tensor(mask2[:], l2[:], bc3(m2),
                                        op=mybir.AluOpType.is_equal)
                mv2 = rt3("mv2")
                nc.vector.tensor_tensor(mv2[:], mask2[:], rev3,
                                        op=mybir.AluOpType.mult)
                sel2 = rt1("sel2")
                nc.vector.reduce_max(sel2[:], mv2[:], axis=mybir.AxisListType.X)
                m2f = rt3("m2f")
                nc.vector.tensor_tensor(m2f[:], mv2[:], bc3(sel2),
                                        op=mybir.AluOpType.is_equal)
                dlt = rt1("dlt")
                nc.vector.tensor_sub(dlt[:], m2[:], m1[:])
                ex = rt1("ex")
                nc.scalar.activation(ex[:], dlt[:],
                                     mybir.ActivationFunctionType.Exp)
                den = rt1("den")
                nc.vector.tensor_scalar_add(den[:], ex[:], 1.0)
                w1t = rt1("w1t")
                nc.vector.reciprocal(w1t[:], den[:])
                w2t = rt1("w2t")
                nc.vector.tensor_mul(w2t[:], ex[:], w1t[:])
                comb3 = rt3("comb3")
                nc.vector.tensor_tensor(comb3[:], m1f[:], bc3(w1t),
                                        op=mybir.AluOpType.mult)
                m2fw = rt3("m2fw")
                nc.vector.tensor_tensor(m2fw[:], m2f[:], bc3(w2t),
                                        op=mybir.AluOpType.mult)
                nc.vector.tensor_add(comb3[:], comb3[:], m2fw[:])
                sel3 = rt3("sel3")
                nc.vector.tensor_add(sel3[:], m1f[:], m2f[:])
                wh = rtp.tile([P, NTCH, 2], F16, tag="wh", name="wh")
                nc.vector.tensor_copy(wh[:, :, 0], w1t[:, :, 0])
                nc.vector.tensor_copy(wh[:, :, 1], w2t[:, :, 0])

                # comb feature-major rows (fp16) for the rb2/sb2 fold
                cr8 = rtp.tile([8, TC], F16, tag="cr8", name="cr8")
                nc.vector.memset(cr8[:], 1.0)  # row 7 stays all-ones
                for tch in range(NTCH):
                    pt2 = pssp.tile([E, P], F32, tag="ps", name="pt2")
                    nc.tensor.transpose(pt2[:], comb3[:, tch, :], ident[:])
                    nc.vector.tensor_copy(cr8[0:E, tch * P:(tch + 1) * P], pt2[:])

                # ---------- slot assignment (cumsum via PE) ----------
                slotFM = rtp.tile([E, TC], F32, tag="efm", name="slotFM")
                for h in range(2):
                    pos_ps = pssp.tile([E, 512], F32, tag="ps", name=f"pos{h}")
                    for tbl in range(4):
                        tb = h * 4 + tbl
                        for tpb in range(tb + 1):
                            rhs = (trio_sb[:, 0:P] if tpb == tb
                                   else trio_sb[:, P:2 * P])
                            nc.tensor.matmul(
                                pos_ps[:, tbl * P:(tbl + 1) * P],
                                sel3[:, tpb, :],
                                rhs,
                                start=(tpb == 0), stop=(tpb == tb),
                            )
                    nc.vector.tensor_scalar_add(
                        slotFM[:, h * 512:(h + 1) * 512], pos_ps[:], capb_sb[:])
                slot3 = rt3("slot3")
                for tch in range(NTCH):
                    pt3 = pssp.tile([P, E], F32, tag="ps", name="pt3")
                    nc.tensor.transpose(pt3[:], slotFM[:, tch * P:(tch + 1) * P],
                                        ident[0:E, 0:E])
                    nc.vector.tensor_copy(slot3[:, tch, :], pt3[:])
                tt = rt3("tt")
                nc.vector.tensor_tensor(tt[:], m1f[:], slot3[:],
                                        op=mybir.AluOpType.mult)
                slot1 = rt1("slot1")
                nc.vector.reduce_sum(slot1[:], tt[:], axis=mybir.AxisListType.X)
                nc.vector.tensor_tensor(tt[:], m2f[:], slot3[:],
                                        op=mybir.AluOpType.mult)
                slot2 = rt1("slot2")
                nc.vector.reduce_sum(slot2[:], tt[:], axis=mybir.AxisListType.X)

                s12 = idxp.tile([P, 2, NTCH], I16, tag="s12")
                nc.vector.tensor_copy(s12[:, 0, :], slot1[:, :, 0])
                nc.vector.tensor_copy(s12[:, 1, :], slot2[:, :, 0])
                nc.gpsimd.dma_start(
                    lscr.rearrange("k (c p) -> p k c", p=P), s12[:])
                wr = idxp.tile([P, 2, TC // 16], I16, tag="wr")
                for r in range(8):
                    nc.gpsimd.dma_start(
                        wr[r * 16:(r + 1) * 16, :, :],
                        lscr.rearrange("k (c q) -> q k c", q=16))

                # ---------- dispatch scatters (SWDGE) ----------
                for k in range(2):
                    nc.gpsimd.dma_scatter_add(
                        xdisp[:, :], xtk[:], wr[:, k, :],
                        TC, TC, D,
                    )

                # tail-only constants (issued late, off the critical path)
                rbf = finp.tile([E, D], F32, tag="fin", name="rbf")
                nc.sync.dma_start(rbf[:], rb2[:])
                sbf = finp.tile([1, D], F32, tag="fin", name="sbf")
                nc.sync.dma_start(sbf[:], sb2[None, :])
                cb8 = constp.tile([8, D], F16, tag="cb8")
                nc.vector.tensor_copy(cb8[0:E, :], rbf[:])
                sbh = constp.tile([1, D], F16, tag="sbh")
                nc.vector.tensor_copy(sbh[:], sbf[:])
                nc.sync.dma_start(cb8[E:E + 1, :], sbh[0:1, :])

                # ---------- shared expert L1 (fp16, feature-major) ----------
                hsh = hshp.tile([P, HC, TC], F16, tag="hsh")
                WCH = 256
                for cg in range(2):
                    wt = swt[cg]
                    for cil in range(2):
                        for hl in range(WCH // P):
                            hcc = (2 * cg + cil) * 2 + hl
                            ph = psbp.tile([P, 2, 512], F32, tag="psb")
                            for dc in range(DC):
                                for tb in range(2):
                                    nc.tensor.matmul(
                                        ph[:, tb, :],
                                        wt[:, cil, dc, hl * P:(hl + 1) * P],
                                        xr[:, dc, tb * 512:(tb + 1) * 512],
                                        start=(dc == 0), stop=(dc == DC - 1),
                                    )
                            nc.scalar.activation(
                                hsh[:, hcc, :],
                                ph[:].rearrange("p a b -> p (a b)"),
                                act_fn, bias=sb1_sb[:, hcc:hcc + 1])
                # ---------- routed experts ----------
                for e in range(E):
                    # one xbar-transpose load for the whole expert block:
                    # xdisp columns are stored permuted (col k <-> feature
                    # (k%8)*128 + k//8) so the transposed [128, 8, CAP] output
                    # lands exactly feature-major.
                    xg = xgp.tile([P, DC, CAP], F16, tag="xg")
                    nc.sync.dma_start_transpose(
                        xg[:], xdisp[e * CAP:(e + 1) * CAP, :])
                    hbuf = hp.tile([P, HC, CAP], F16, tag="h")
                    for cg in range(2):
                        wt = w1p.tile([P, 2, DC, WCH], F16, tag="w1")
                        nc.sync.dma_start(wt[:], rw1q[e][:, 2 * cg:2 * cg + 2,
                                                        :, :])
                        for cil in range(2):
                            for hl in range(WCH // P):
                                hcc = (2 * cg + cil) * 2 + hl
                                ph = pssp.tile([P, CAP], F32, tag="ps",
                                               name="ph")
                                for dc in range(DC):
                                    nc.tensor.matmul(
                                        ph[:],
                                        wt[:, cil, dc, hl * P:(hl + 1) * P],
                                        xg[:, dc, :],
                                        start=(dc == 0), stop=(dc == DC - 1),
                                    )
                                nc.scalar.activation(
                                    hbuf[:, hcc, :], ph[:],
                                    act_fn, bias=rb1_sb[:, e, hcc:hcc + 1])
                    # L2: slot-major, stationary = h slices, moving = rw2 rows
                    w2f = w2p.tile([P, HC, D], F16, tag="w2")
                    nc.sync.dma_start(w2f[:], rw2q[e][:, :, :])
                    pos = [psbp.tile([P, 2, 512], F32, tag="psb",
                                     name=f"pos{e}_{i}")
                           for i in range(CAP // P)]
                    for hcc in range(HC):
                        for sb_ in range(CAP // P):
                            for nh in range(2):
                                nc.tensor.matmul(
                                    pos[sb_][:, nh, :],
                                    hbuf[:, hcc, sb_ * P:(sb_ + 1) * P],
                                    w2f[:, hcc, nh * 512:(nh + 1) * 512],
                                    start=(hcc == 0), stop=(hcc == HC - 1),
                                )
                    eos = eosp.tile([P, CAP // P, D], F16, tag="eos")
                    for sb_ in range(CAP // P):
                        nc.vector.tensor_copy(
                            eos[:, sb_, :],
                            pos[sb_][:].rearrange("p a b -> p (a b)"))
                    nc.sync.dma_start(
                        eoh_t[:, e * (CAP // P):(e + 1) * (CAP // P), :], eos[:])

                # ---------- combine gathers (non-transpose, HBM source) ----------
                cc = [None, None]
                for k in range(2):
                    cc[k] = tkcp.tile([P, NTCH, D], F16, tag="tkc",
                                      name=f"cc{k}")
                    nc.gpsimd.dma_gather(
                        cc[k][:], eoh[:, :], wr[:, k, :],
                        TC, TC, D,
                    )

                # ---------- shared L2 (token-major) + final + out ----------
                # 1-bank psum tiles so this never waits on the expert-L2 psum
                # rotation; fin split per 512-wide half for pipelining.
                # shared L2 decoupled from the gather-gated fin chain: stage
                # to SBUF (reusing xr's pool slot) so PE streams all 8 blocks
                # while the combine gathers' descriptor-gen runs on Q7.
                shout = xrp.tile([P, NTCH, D], F16, tag="xr", name="shout")
                for tb in range(NTCH):
                    for nh in range(2):
                        po = pssp.tile([P, 512], F32, tag="ps",
                                       name=f"po{tb}_{nh}")
                        for hcc in range(HC):
                            nc.tensor.matmul(
                                po[:],
                                hsh[:, hcc, tb * P:(tb + 1) * P],
                                sw2f[:, hcc, nh * 512:(nh + 1) * 512],
                                start=(hcc == 0), stop=False,
                            )
                        nc.tensor.matmul(
                            po[:],
                            cr8[:, tb * P:(tb + 1) * P],
                            cb8[:, nh * 512:(nh + 1) * 512],
                            start=False, stop=True,
                        )
                        nc.scalar.activation(
                            shout[:, tb, nh * 512:(nh + 1) * 512], po[:],
                            mybir.ActivationFunctionType.Copy)
                for tb in range(NTCH):
                    fin = finp.tile([P, D], F32, tag="fin")
                    for nh in range(2):
                        sl = slice(nh * 512, (nh + 1) * 512)
                        nc.vector.tensor_tensor(
                            fin[:, sl], cc[0][:, tb, sl],
                            wh[:, tb, 0:1].to_broadcast((P, 512)),
                            op=mybir.AluOpType.mult)
                        fin2 = finp.tile([P, 512], F32, tag="fin2")
                        nc.scalar.activation(
                            fin2[:], cc[1][:, tb, sl],
                            mybir.ActivationFunctionType.Copy,
                            scale=w2t[:, tb, 0:1])
                        nc.vector.tensor_add(fin[:, sl], fin[:, sl], fin2[:])
                    nc.vector.tensor_add(
                        fin[:], fin[:],
                        shout[:, tb, :])
                    nc.sync.dma_start(outt_t[:, tb, :], fin[:])

            if n_reps == 1:
                body()
            else:
                tc.For_i_unrolled(0, n_reps, 1, body, max_unroll=1)

    nc.compile()
    return nc


class Runner:
    """Compile once, dispatch many times (axon/PJRT path)."""

    def __init__(self, nc):
        import jax
        from jax.sharding import Mesh, PartitionSpec
        from jax.experimental.shard_map import shard_map
        from concourse import bass2jax

        bass2jax.install_neuronx_cc_hook()
        self.nc = nc
        self.jax = jax
        pname = nc.partition_id_tensor.name if nc.partition_id_tensor else None
        in_names, out_names, out_avals = [], [], []
        for alloc in nc.m.functions[0].allocations:
            if not isinstance(alloc, mybir.MemoryLocationSet):
                continue
            name = alloc.memorylocations[0].name
            if alloc.kind == "ExternalInput":
                if name != pname:
                    in_names.append(name)
            elif alloc.kind == "ExternalOutput":
                out_names.append(name)
                out_avals.append(jax.core.ShapedArray(
                    tuple(alloc.tensor_shape), mybir.dt.np(alloc.dtype)))
        self.in_names, self.out_names, self.out_avals = in_names, out_names, out_avals
        all_names = in_names + out_names + ([pname] if pname else [])

        def _body(*args):
            operands = list(args)
            if pname is not None:
                operands.append(bass2jax.partition_id_tensor())
            outs = bass2jax._bass_exec_p.bind(
                *operands,
                out_avals=tuple(out_avals),
                in_names=tuple(all_names),
                out_names=tuple(out_names),
                lowering_input_output_aliases=(),
                sim_require_finite=True, sim_require_nnan=True, nc=nc)
            return tuple(outs)

        devices = jax.devices()[:N_CORES]
        mesh = Mesh(np.asarray(devices), ("core",))
        nin = len(in_names) + len(out_names)
        self.fn = jax.jit(
            shard_map(_body, mesh=mesh,
                      in_specs=(PartitionSpec("core"),) * nin,
                      out_specs=(PartitionSpec("core"),) * len(out_names),
                      check_rep=False),
            keep_unused=True)

    def concat_inputs(self, in_maps):
        args = []
        for name in self.in_names:
            args.append(np.concatenate([m[name] for m in in_maps], axis=0))
        for av in self.out_avals:
            args.append(np.zeros((N_CORES * av.shape[0],) + av.shape[1:],
                                 av.dtype))
        return args

    def __call__(self, args):
        outs = self.fn(*args)
        self.jax.block_until_ready(outs)
        return outs

    def split_outputs(self, outs):
        res = []
        for c in range(N_CORES):
            d = {}
            for i, name in enumerate(self.out_names):
                a = np.asarray(outs[i])
                d[name] = a.reshape(N_CORES, *self.out_avals[i].shape)[c]
            res.append(d)
        return res


_RUNNER_CACHE = {}


def get_runner(n_reps=1, act_fn=None):
    key = (n_reps, act_fn)
    if key not in _RUNNER_CACHE:
        _RUNNER_CACHE[key] = Runner(build_nc(n_reps, act_fn=act_fn))
    return _RUNNER_CACHE[key]


# xdisp column k holds feature (k%8)*128 + k//8, so the single xbar-transpose
# load of an expert block lands feature-major as [128(p), 8(c), CAP].
_XPERM = np.arange(D)  # identity: xbar 3D out flattens col = c*128+p


def make_in_maps(x, gate_w, expert_bias, sw1, sb1, sw2, sb2, rw1, rb1, rw2, rb2):
    xf = np.ascontiguousarray(np.asarray(x, dtype=np.float32).reshape(T, D))
    gwT = np.ascontiguousarray(np.asarray(gate_w, np.float32).T)
    rev = np.arange(E, 0, -1, dtype=np.float32)
    tri = np.triu(np.ones((P, P), np.float32), k=1)
    trio = np.concatenate([tri, np.ones((P, P), np.float32)], axis=1)
    capb = np.arange(E, dtype=np.float32) * CAP

    def h16(a):
        return np.ascontiguousarray(np.asarray(a, np.float32).astype(np.float16))

    def f32(a):
        return np.ascontiguousarray(np.asarray(a, np.float32))

    sw1_16 = np.asarray(sw1, np.float32).astype(np.float16)
    sw2_16 = np.asarray(sw2, np.float32).astype(np.float16)
    rw1_16 = np.asarray(rw1, np.float32).astype(np.float16)
    rw2_16 = np.asarray(rw2, np.float32).astype(np.float16)
    shared = {
        "gwT": gwT, "eb": f32(expert_bias), "rev7": rev,
        "trio": np.ascontiguousarray(trio), "capb": capb,
        "sw1q": np.ascontiguousarray(
            sw1_16.reshape(8, P, 4, 256).transpose(1, 2, 0, 3)),
        "sb1": f32(sb1),
        "sw2q": np.ascontiguousarray(
            sw2_16.reshape(8, P, D).transpose(1, 0, 2)),
        "sb2": f32(sb2),
        "rw1q": np.ascontiguousarray(
            rw1_16.reshape(E, 8, P, 4, 256).transpose(0, 2, 3, 1, 4)),
        "rb1": f32(rb1),
        "rw2q": np.ascontiguousarray(
            rw2_16.reshape(E, 8, P, D).transpose(0, 2, 1, 3)),
        "rb2": f32(rb2),
    }
    in_maps = []
    for c in range(N_CORES):
        xs = xf[c * TC:(c + 1) * TC, :]
        in_maps.append({
            "xTp": np.ascontiguousarray(
                xs.T.reshape(8, P, TC).transpose(1, 0, 2)),
            "xtokp": np.ascontiguousarray(
                h16(xs)[:, _XPERM].reshape(8, P, D).transpose(1, 0, 2)),
            **shared,
        })
    return in_maps


def kernel(x, gate_w, expert_bias, sw1, sb1, sw2, sb2, rw1, rb1, rw2, rb2):
    runner = get_runner(1)
    in_maps = make_in_maps(x, gate_w, expert_bias, sw1, sb1, sw2, sb2,
                           rw1, rb1, rw2, rb2)
    outs = runner(runner.concat_inputs(in_maps))
    res = runner.split_outputs(outs)
    parts = [res[c]["outt"] for c in range(N_CORES)]
    out = np.concatenate(parts, axis=0).reshape(B, S, D)
    return np.ascontiguousarray(out.astype(np.float32))
